# revision 1
# baseline (speedup 1.0000x reference)
"""DenseGINConv on 8 TRN2 NeuronCores (v3: fp16 gather/aggregation path).

  agg = segment_sum(x[edge_src], edge_dst, N)        # gather + scatter-add
  h   = (1+eps)*x + agg
  out = relu(relu(relu(h @ W1 + b1) @ W2 + b2) + bias)

Strategy (fully SPMD, zero collectives):
  - Shard edges by dst range: core i owns dst nodes [i*12500, (i+1)*12500).
  - Replicate x as an fp32 gather table in every core's HBM (input staging is
    free); gather src rows with the dma_gather GPSIMD ucode, which needs
    int16 indices -> the table is split into 4 chunks of <=32768 rows and each
    block's edges are grouped by chunk.
  - Per core, 98 dst-blocks of 128 dst slots. Per (block, chunk) the edge
    count is padded to a multiple of 128 (pad edges gather row 0 of the chunk
    and carry dst slot -1, which zeroes their one-hot column).
  - Per block: gather lands [128 edges x 128 ch] chunk-columns; a one-hot
    (edge -> dst slot) built on the vector engine turns the segment-sum into
    PE matmuls accumulating in PSUM, producing agg^T [C, 128 slots] directly
    in the transposed orientation the MLP wants (x @ W == (W^T x^T)^T).
  - MLP per block with W1/W2 as stationary lhsT (their [in, out] layout is
    already transposed), biases as per-partition activation bias. Output is
    written transposed [C, nodes]; the host transposes back.

The gather dominates; fp16 rows (256B) halve its traffic vs fp32 and the
dma_gather ucode sustains ~100+ GB/s/core under 8-core contention. The MLP
and the (1+eps)x add stay fp32, so end-to-end error vs the fp32 reference is
just the fp16 quantization of gathered x (~2e-4 max-rel).
"""

import math

import numpy as np

import concourse.bacc as bacc
import concourse.mybir as mybir
import concourse.tile as tile
from concourse.bass_utils import run_bass_kernel_spmd
from concourse.library_config import mlp as mlp_lib

N = 100000
C = 128
M = 8            # cores
NPC = N // M     # nodes per core = 12500
BLK = 128                       # dst slots per block (psum free dim)
NBLK = 104                      # dst blocks / core (6.5% slack for balancing)
SLOTS = NBLK * BLK              # padded dst slots / core
P = 128
NCH = 4                         # x-table chunks (int16 index range)
CH = math.ceil((N + 1) / NCH)   # rows per chunk (25001 <= 32768)
MAXCOLS_CALL = 7                # <=7*128 idxs per dma_gather (64-desc rings)

f32 = mybir.dt.float32
f16 = mybir.dt.float16
i16 = mybir.dt.int16

_cache = {}


def build(Mmat):
    """Build the per-core Bass program. Mmat[b][c] = 128-edge columns for
    (dst-block b, x-chunk c); identical across cores."""
    nc = bacc.Bacc(
        "TRN2", target_bir_lowering=False, debug=False, enable_asserts=True,
        num_swdge_queues=4,
    )
    totcol = int(sum(sum(r) for r in Mmat))
    sum16 = totcol * 8  # idx columns (int16, 16-wrapped): 128/16 per column

    xt = nc.dram_tensor("xt", [NCH * CH, C], f16, kind="ExternalInput")
    srcs = nc.dram_tensor("srcs", [P, sum16], i16, kind="ExternalInput")
    dstl = nc.dram_tensor("dstl", [P, totcol], f16, kind="ExternalInput")
    xsT = nc.dram_tensor("xsT", [P, SLOTS], f32, kind="ExternalInput")
    w1 = nc.dram_tensor("W1", [C, C], f32, kind="ExternalInput")
    w2 = nc.dram_tensor("W2", [C, C], f32, kind="ExternalInput")
    b1 = nc.dram_tensor("b1c", [C, 1], f32, kind="ExternalInput")
    b2 = nc.dram_tensor("b2c", [C, 1], f32, kind="ExternalInput")
    bias = nc.dram_tensor("biasc", [C, 1], f32, kind="ExternalInput")
    iota = nc.dram_tensor("iota", [P, BLK], f16, kind="ExternalInput")
    outT = nc.dram_tensor("outT", [P, SLOTS], f32, kind="ExternalOutput")

    maxblk = max(sum(r) for r in Mmat)

    with tile.TileContext(nc) as tc:
        with (
            tc.tile_pool(name="const", bufs=1) as cp,
            tc.tile_pool(name="gath", bufs=4) as gp,
            tc.tile_pool(name="oh", bufs=2) as op,
            tc.tile_pool(name="mlp", bufs=3) as mp,
            tc.tile_pool(name="psA", bufs=2, space="PSUM") as psA,
            tc.tile_pool(name="psB", bufs=2, space="PSUM") as psB,
            tc.tile_pool(name="psC", bufs=2, space="PSUM") as psC,
        ):
            nc.gpsimd.load_library(mlp_lib)
            srcs_sb = cp.tile([P, sum16], i16)
            nc.sync.dma_start(srcs_sb[:], srcs[:])
            dstl_sb = cp.tile([P, totcol], f16)
            nc.sync.dma_start(dstl_sb[:], dstl[:])
            xsT_sb = cp.tile([P, SLOTS], f32)
            nc.sync.dma_start(xsT_sb[:], xsT[:])
            w1_sb = cp.tile([C, C], f32)
            nc.sync.dma_start(w1_sb[:], w1[:])
            w2_sb = cp.tile([C, C], f32)
            nc.sync.dma_start(w2_sb[:], w2[:])
            b1_sb = cp.tile([C, 1], f32)
            nc.sync.dma_start(b1_sb[:], b1[:])
            b2_sb = cp.tile([C, 1], f32)
            nc.sync.dma_start(b2_sb[:], b2[:])
            bias_sb = cp.tile([C, 1], f32)
            nc.sync.dma_start(bias_sb[:], bias[:])
            iota_sb = cp.tile([P, BLK], f16)
            nc.sync.dma_start(iota_sb[:], iota[:])

            col = 0      # global gbuf/one-hot column counter
            seg16 = 0    # idx column counter
            qn = 0
            for b in range(NBLK):
                mb = int(sum(Mmat[b]))
                if mb == 0:
                    continue
                gb = gp.tile([P, maxblk * C], f16, tag="g")
                coff = 0
                for c in range(NCH):
                    mc = int(Mmat[b][c])
                    while mc > 0:
                        mk = min(mc, MAXCOLS_CALL)
                        ni = mk * 128
                        nc.gpsimd.dma_gather(
                            gb[:, coff * C:(coff + mk) * C].rearrange(
                                "p (k e) -> p k e", e=C
                            ),
                            xt[c * CH:(c + 1) * CH, :],
                            srcs_sb[:, seg16:seg16 + ni // 16],
                            ni, ni, C, queue_num=qn % 4,
                        )
                        qn += 1
                        seg16 += ni // 16
                        coff += mk
                        mc -= mk
                oh = op.tile([P, maxblk * BLK], f16, tag="oh")
                nc.vector.tensor_tensor(
                    out=oh[:, :mb * BLK].rearrange("p (m e) -> p m e", e=BLK),
                    in0=dstl_sb[:, col:col + mb]
                    .rearrange("p (m o) -> p m o", o=1)
                    .to_broadcast([P, mb, BLK]),
                    in1=iota_sb[:]
                    .rearrange("p (o e) -> p o e", o=1)
                    .to_broadcast([P, mb, BLK]),
                    op=mybir.AluOpType.is_equal,
                )
                agg = psA.tile([P, BLK], f32, tag="agg")
                for j in range(mb):
                    nc.tensor.matmul(
                        out=agg[:],
                        lhsT=gb[:, j * C:(j + 1) * C],
                        rhs=oh[:, j * BLK:(j + 1) * BLK],
                        start=(j == 0),
                        stop=(j == mb - 1),
                    )
                bcols = slice(b * BLK, (b + 1) * BLK)
                hT = mp.tile([P, BLK], f32, tag="hT")
                nc.vector.tensor_add(out=hT[:], in0=agg[:], in1=xsT_sb[:, bcols])
                ps1 = psB.tile([P, BLK], f32, tag="ps1")
                nc.tensor.matmul(
                    out=ps1[:], lhsT=w1_sb[:], rhs=hT[:], start=True, stop=True
                )
                h1 = mp.tile([P, BLK], f32, tag="h1")
                nc.scalar.activation(
                    h1[:], ps1[:], mybir.ActivationFunctionType.Relu, bias=b1_sb[:]
                )
                ps2 = psC.tile([P, BLK], f32, tag="ps2")
                nc.tensor.matmul(
                    out=ps2[:], lhsT=w2_sb[:], rhs=h1[:], start=True, stop=True
                )
                h2 = mp.tile([P, BLK], f32, tag="h2")
                nc.scalar.activation(
                    h2[:], ps2[:], mybir.ActivationFunctionType.Relu, bias=b2_sb[:]
                )
                ob = mp.tile([P, BLK], f32, tag="ob")
                nc.scalar.activation(
                    ob[:], h2[:], mybir.ActivationFunctionType.Relu, bias=bias_sb[:]
                )
                nc.sync.dma_start(out=outT[:, bcols], in_=ob[:])
                col += mb

    nc.compile()
    return nc


def prep(x, edge_src, edge_dst, eps):
    """Host-side sharding -> per-core (srcs16, dstl, xsT) + shared table/M."""
    x = np.asarray(x, dtype=np.float32)
    edge_src = np.asarray(edge_src).astype(np.int64)
    edge_dst = np.asarray(edge_dst).astype(np.int64)
    epsv = float(np.asarray(eps).reshape(-1)[0])

    core = edge_dst // NPC
    dst_local = edge_dst - core * NPC
    chunk = edge_src // CH
    lidx = (edge_src - chunk * CH).astype(np.int16)

    percore = []
    pos_list = []
    counts = np.zeros((M, NBLK, NCH), dtype=np.int64)
    for i in range(M):
        sel = core == i
        dl, c_i = dst_local[sel], chunk[sel]
        # per-dst degree per chunk, then balance dsts into blocks so each
        # (block, chunk) stays <= 4*128 edges (uniform M=4 across cores)
        deg = np.bincount(dl * NCH + c_i, minlength=NPC * NCH).reshape(NPC, NCH)
        caps = np.full((NBLK, NCH), 4 * 128, dtype=np.int64)
        dblk, dslot = _balance(deg, NBLK, BLK, caps)
        pos_list.append(dblk * BLK + dslot)
        b_i = dblk[dl]
        slot_i = dslot[dl]
        order = np.lexsort((c_i, b_i))
        percore.append((lidx[sel][order], slot_i[order],
                        b_i[order], c_i[order]))
        cnt = np.bincount(b_i * NCH + c_i, minlength=NBLK * NCH)
        counts[i] = cnt.reshape(NBLK, NCH)

    Mmat = np.ceil(counts.max(axis=0) / 128).astype(np.int64)  # [NBLK, NCH]
    totcol = int(Mmat.sum())

    # column-start offset of each (b, c) in the concatenated layout
    flat = Mmat.reshape(-1)
    colstart = np.zeros(NBLK * NCH, dtype=np.int64)
    colstart[1:] = np.cumsum(flat)[:-1]

    srcs_list, dstl_list, xsT_list = [], [], []
    for i in range(M):
        li, sl, b_i, c_i = percore[i]
        # position of each edge inside its (b,c) group
        key = b_i * NCH + c_i
        kcnt = counts[i].reshape(-1)
        kstart = np.zeros(NBLK * NCH, dtype=np.int64)
        kstart[1:] = np.cumsum(kcnt)[:-1]
        pos = np.arange(len(li)) - kstart[key]
        gpos = colstart[key] * 128 + pos  # position in padded edge stream

        v = np.zeros(totcol * 128, dtype=np.int16)   # pad: row 0 of chunk
        d = np.full(totcol * 128, -1.0, dtype=np.float16)
        v[gpos] = li
        d[gpos] = sl

        # idx stream wraps per 16 within each call; calls are (b,c) pieces of
        # <=MAXCOLS_CALL columns. Wrapping j -> [j%16, j//16] is position-local
        # per call, but since calls are whole columns and 128 % 16 == 0, the
        # wrap is identical whether done per call or over the whole stream.
        w = v.reshape(-1, 16).T.copy()               # [16, totcol*8]
        srcs_list.append(np.tile(w, (8, 1)))
        dstl_list.append(
            np.ascontiguousarray(d.reshape(totcol, 128).T)  # [128, totcol]
        )
        xs = np.zeros((P, SLOTS), dtype=np.float32)
        xs[:, pos_list[i]] = ((1.0 + epsv) * x[i * NPC:(i + 1) * NPC]).T
        xsT_list.append(xs)

    xt = np.zeros((NCH * CH, C), dtype=np.float16)
    xt[:N] = x
    return Mmat, srcs_list, dstl_list, xsT_list, xt, pos_list


def _balance(deg, nbins, cap_slots, cap_edges):
    """Best-fit-decreasing: assign dsts (rows of deg [ND, NCH]) to nbins
    blocks, <= cap_slots dsts and (soft) <= cap_edges[b, c] edges each."""
    nd = deg.shape[0]
    tot = deg.sum(axis=1)
    order = np.argsort(-tot, kind="stable")
    sums = np.zeros((nbins, deg.shape[1]), dtype=np.int64)
    load = np.zeros(nbins, dtype=np.int64)
    cnt = np.zeros(nbins, dtype=np.int64)
    blk = np.empty(nd, dtype=np.int64)
    slot = np.empty(nd, dtype=np.int64)
    big = 1 << 50
    for d in order:
        v = deg[d]
        ok = (cnt < cap_slots) & ((sums + v) <= cap_edges).all(axis=1)
        if ok.any():
            b = int(np.argmin(np.where(ok, load, big)))  # LPT: least-loaded fit
        else:
            over = np.maximum(sums + v - cap_edges, 0).sum(axis=1)
            over[cnt >= cap_slots] = big
            b = int(np.argmin(over))
        blk[d] = b
        slot[d] = cnt[b]
        cnt[b] += 1
        load[b] += tot[d]
        sums[b] += v
    return blk, slot


def make_in_maps(inputs):
    Mmat, srcs_list, dstl_list, xsT_list, xt, pos_list = prep(
        inputs["x"], inputs["edge_src"], inputs["edge_dst"], inputs["eps"]
    )
    w1 = np.ascontiguousarray(np.asarray(inputs["W1"], dtype=np.float32))
    w2 = np.ascontiguousarray(np.asarray(inputs["W2"], dtype=np.float32))
    b1c = np.asarray(inputs["b1"], dtype=np.float32).reshape(C, 1)
    b2c = np.asarray(inputs["b2"], dtype=np.float32).reshape(C, 1)
    biasc = np.asarray(inputs["bias"], dtype=np.float32).reshape(C, 1)
    iota = np.tile(np.arange(BLK, dtype=np.float16), (P, 1))
    in_maps = [
        dict(
            xt=xt, srcs=srcs_list[i], dstl=dstl_list[i], xsT=xsT_list[i],
            W1=w1, W2=w2, b1c=b1c, b2c=b2c, biasc=biasc, iota=iota,
        )
        for i in range(M)
    ]
    return Mmat, in_maps, pos_list


def get_program(Mmat):
    key = Mmat.tobytes()
    if key not in _cache:
        _cache[key] = build(Mmat)
    return _cache[key]


def assemble(results, pos_list):
    out = np.empty((N, C), dtype=np.float32)
    for i in range(M):
        out[i * NPC:(i + 1) * NPC] = results[i]["outT"].T[pos_list[i]]
    return out


def kernel(**inputs) -> np.ndarray:
    Mmat, in_maps, pos_list = make_in_maps(inputs)
    nc = get_program(Mmat)
    last_err = None
    for _ in range(3):  # rare transient NRT_EXEC_UNIT_UNRECOVERABLE flakes
        try:
            res = run_bass_kernel_spmd(nc, in_maps, list(range(M)))
            return assemble(res.results, pos_list)
        except Exception as e:  # noqa: BLE001
            last_err = e
    raise last_err



# revision 2
# speedup vs baseline: 1.3759x; 1.3759x over previous
"""DenseGINConv on 8 TRN2 NeuronCores (v4: 512B pair-overfetch gather).

  agg = segment_sum(x[edge_src], edge_dst, N)        # gather + scatter-add
  h   = (1+eps)*x + agg
  out = relu(relu(relu(h @ W1 + b1) @ W2 + b2) + bias)

Strategy (fully SPMD, zero collectives):
  - Shard edges by dst range: core i owns dst nodes [i*12500, (i+1)*12500).
  - Replicate x as a PAIR table in every core's HBM: row i of the table is
    [x[i], x[i+1]] (512B in fp16). Each gathered descriptor is 512B — the
    DMA bus moves 512B granules anyway, so fetching the pair costs the same
    as one 256B row, and the matmul simply uses the first half.
  - dma_gather needs int16 indices -> 4 chunks of 25001 rows.
  - Dst-blocks of 128 slots are grouped into superblocks of SBK blocks; one
    gather call covers a whole (superblock, chunk) group, cutting the Pool
    engine's fixed SWDGE overhead (994ns/call) vs per-(block,chunk) calls.
    dynamic_dma_scratch_size=32768 doubles the descriptor rings to allow
    the bigger calls.
  - Per block: one-hot (edge -> dst slot) built on the vector engine turns
    the segment-sum into PE matmuls accumulating in PSUM, producing agg^T
    [C, 128 slots] directly in the transposed orientation the MLP wants.
  - MLP per block with W1/W2 as stationary lhsT, biases as per-partition
    activation bias. Output is written transposed [C, nodes]; the host
    transposes back.
"""

import math

import numpy as np

import concourse.bacc as bacc
import concourse.mybir as mybir
import concourse.tile as tile
from concourse.bass_utils import run_bass_kernel_spmd
from concourse.library_config import mlp as mlp_lib

N = 100000
C = 128
M = 8            # cores
NPC = N // M     # nodes per core = 12500
BLK = 128                       # dst slots per block (psum free dim)
NBLK = 104                      # dst blocks / core (6.5% slack for balancing)
SBK = 2                         # blocks per superblock (gather-call granule)
NSB = NBLK // SBK
SLOTS = NBLK * BLK              # padded dst slots / core
P = 128
NCH = 4                         # x-table chunks (int16 index range)
CH = math.ceil((N + 1) / NCH)   # rows per chunk (25001 <= 32768)
MAXCOLS_CALL = 14               # <=14*128 idxs per dma_gather (128-desc rings)

f32 = mybir.dt.float32
f16 = mybir.dt.float16
i16 = mybir.dt.int16

_cache = {}


def _key(b, c):
    return ((b // SBK) * NCH + c) * SBK + (b % SBK)


def build(Mmat):
    """Build the per-core Bass program. Mmat[b][c] = 128-edge columns for
    (dst-block b, x-chunk c); identical across cores."""
    nc = bacc.Bacc(
        "TRN2", target_bir_lowering=False, debug=False, enable_asserts=True,
        num_swdge_queues=4, dynamic_dma_scratch_size=32768,
    )
    totcol = int(sum(sum(r) for r in Mmat))
    sum16 = totcol * 8  # idx columns (int16, 16-wrapped): 128/16 per column

    xt = nc.dram_tensor("xt", [NCH * CH, 2 * C], f16, kind="ExternalInput")
    srcs = nc.dram_tensor("srcs", [P, sum16], i16, kind="ExternalInput")
    dstl = nc.dram_tensor("dstl", [P, totcol], f16, kind="ExternalInput")
    xsT = nc.dram_tensor("xsT", [P, SLOTS], f32, kind="ExternalInput")
    w1 = nc.dram_tensor("W1", [C, C], f32, kind="ExternalInput")
    w2 = nc.dram_tensor("W2", [C, C], f32, kind="ExternalInput")
    b1 = nc.dram_tensor("b1c", [C, 1], f32, kind="ExternalInput")
    b2 = nc.dram_tensor("b2c", [C, 1], f32, kind="ExternalInput")
    bias = nc.dram_tensor("biasc", [C, 1], f32, kind="ExternalInput")
    iota = nc.dram_tensor("iota", [P, BLK], f16, kind="ExternalInput")
    outT = nc.dram_tensor("outT", [P, SLOTS], f32, kind="ExternalOutput")

    # max columns in one superblock (gather tile sizing)
    maxsb = max(
        sum(int(Mmat[SBK * b2 + s][c]) for s in range(SBK) for c in range(NCH))
        for b2 in range(NSB)
    )

    with tile.TileContext(nc) as tc:
        with (
            tc.tile_pool(name="const", bufs=1) as cp,
            tc.tile_pool(name="gath", bufs=3) as gp,
            tc.tile_pool(name="oh", bufs=2) as op,
            tc.tile_pool(name="mlp", bufs=3) as mp,
            tc.tile_pool(name="psA", bufs=2, space="PSUM") as psA,
            tc.tile_pool(name="psB", bufs=2, space="PSUM") as psB,
            tc.tile_pool(name="psC", bufs=2, space="PSUM") as psC,
        ):
            nc.gpsimd.load_library(mlp_lib)
            srcs_sb = cp.tile([P, sum16], i16)
            nc.sync.dma_start(srcs_sb[:], srcs[:])
            dstl_sb = cp.tile([P, totcol], f16)
            nc.sync.dma_start(dstl_sb[:], dstl[:])
            xsT_sb = cp.tile([P, SLOTS], f32)
            nc.sync.dma_start(xsT_sb[:], xsT[:])
            w1_sb = cp.tile([C, C], f32)
            nc.sync.dma_start(w1_sb[:], w1[:])
            w2_sb = cp.tile([C, C], f32)
            nc.sync.dma_start(w2_sb[:], w2[:])
            b1_sb = cp.tile([C, 1], f32)
            nc.sync.dma_start(b1_sb[:], b1[:])
            b2_sb = cp.tile([C, 1], f32)
            nc.sync.dma_start(b2_sb[:], b2[:])
            bias_sb = cp.tile([C, 1], f32)
            nc.sync.dma_start(bias_sb[:], bias[:])
            iota_sb = cp.tile([P, BLK], f16)
            nc.sync.dma_start(iota_sb[:], iota[:])

            seg16 = 0    # idx column counter
            qn = 0
            col = 0      # global column counter (key order)
            for b2 in range(NSB):
                blocks = [SBK * b2 + s for s in range(SBK)]
                mrun = [[int(Mmat[b][c]) for b in blocks] for c in range(NCH)]
                msb = sum(sum(r) for r in mrun)
                if msb == 0:
                    continue
                gb = gp.tile([P, maxsb * 2 * C], f16, tag="g")
                coff = 0
                for c in range(NCH):
                    mc = sum(mrun[c])
                    while mc > 0:
                        mk = min(mc, MAXCOLS_CALL)
                        ni = mk * 128
                        nc.gpsimd.dma_gather(
                            gb[:, coff * 2 * C:(coff + mk) * 2 * C].rearrange(
                                "p (k e) -> p k e", e=2 * C
                            ),
                            xt[c * CH:(c + 1) * CH, :],
                            srcs_sb[:, seg16:seg16 + ni // 16],
                            ni, ni, 2 * C, queue_num=qn % 4,
                        )
                        qn += 1
                        seg16 += ni // 16
                        coff += mk
                        mc -= mk
                oh = op.tile([P, maxsb * BLK], f16, tag="oh")
                nc.vector.tensor_tensor(
                    out=oh[:, :msb * BLK].rearrange("p (m e) -> p m e", e=BLK),
                    in0=dstl_sb[:, col:col + msb]
                    .rearrange("p (m o) -> p m o", o=1)
                    .to_broadcast([P, msb, BLK]),
                    in1=iota_sb[:]
                    .rearrange("p (o e) -> p o e", o=1)
                    .to_broadcast([P, msb, BLK]),
                    op=mybir.AluOpType.is_equal,
                )
                for s, b in enumerate(blocks):
                    # column positions of block b inside this superblock
                    jcols = []
                    off = 0
                    for c in range(NCH):
                        pre = sum(mrun[c][:s])
                        jcols.extend(
                            range(off + pre, off + pre + mrun[c][s])
                        )
                        off += sum(mrun[c])
                    if not jcols:
                        continue
                    agg = psA.tile([P, BLK], f32, tag="agg")
                    for jj, j in enumerate(jcols):
                        nc.tensor.matmul(
                            out=agg[:],
                            lhsT=gb[:, j * 2 * C:j * 2 * C + C],
                            rhs=oh[:, j * BLK:(j + 1) * BLK],
                            start=(jj == 0),
                            stop=(jj == len(jcols) - 1),
                        )
                    bcols = slice(b * BLK, (b + 1) * BLK)
                    hT = mp.tile([P, BLK], f32, tag="hT")
                    nc.vector.tensor_add(
                        out=hT[:], in0=agg[:], in1=xsT_sb[:, bcols]
                    )
                    ps1 = psB.tile([P, BLK], f32, tag="ps1")
                    nc.tensor.matmul(
                        out=ps1[:], lhsT=w1_sb[:], rhs=hT[:],
                        start=True, stop=True,
                    )
                    h1 = mp.tile([P, BLK], f32, tag="h1")
                    nc.scalar.activation(
                        h1[:], ps1[:], mybir.ActivationFunctionType.Relu,
                        bias=b1_sb[:],
                    )
                    ps2 = psC.tile([P, BLK], f32, tag="ps2")
                    nc.tensor.matmul(
                        out=ps2[:], lhsT=w2_sb[:], rhs=h1[:],
                        start=True, stop=True,
                    )
                    h2 = mp.tile([P, BLK], f32, tag="h2")
                    nc.scalar.activation(
                        h2[:], ps2[:], mybir.ActivationFunctionType.Relu,
                        bias=b2_sb[:],
                    )
                    ob = mp.tile([P, BLK], f32, tag="ob")
                    nc.scalar.activation(
                        ob[:], h2[:], mybir.ActivationFunctionType.Relu,
                        bias=bias_sb[:],
                    )
                    nc.sync.dma_start(out=outT[:, bcols], in_=ob[:])
                col += msb

    nc.compile()
    return nc


def prep(x, edge_src, edge_dst, eps):
    """Host-side sharding -> per-core (srcs16, dstl, xsT) + shared table/M."""
    x = np.asarray(x, dtype=np.float32)
    edge_src = np.asarray(edge_src).astype(np.int64)
    edge_dst = np.asarray(edge_dst).astype(np.int64)
    epsv = float(np.asarray(eps).reshape(-1)[0])

    core = edge_dst // NPC
    dst_local = edge_dst - core * NPC
    chunk = edge_src // CH
    lidx = (edge_src - chunk * CH).astype(np.int16)

    percore = []
    pos_list = []
    counts = np.zeros((M, NBLK, NCH), dtype=np.int64)
    for i in range(M):
        sel = core == i
        dl, c_i = dst_local[sel], chunk[sel]
        # per-dst degree per chunk, then balance dsts into blocks so each
        # (block, chunk) stays <= 4*128 edges (uniform M=4 across cores)
        deg = np.bincount(dl * NCH + c_i, minlength=NPC * NCH).reshape(NPC, NCH)
        caps = np.full((NBLK, NCH), 4 * 128, dtype=np.int64)
        dblk, dslot = _balance(deg, NBLK, BLK, caps)
        pos_list.append(dblk * BLK + dslot)
        b_i = dblk[dl]
        slot_i = dslot[dl]
        key_i = ((b_i // SBK) * NCH + c_i) * SBK + (b_i % SBK)
        order = np.argsort(key_i, kind="stable")
        percore.append((lidx[sel][order], slot_i[order], key_i[order]))
        cnt = np.bincount(b_i * NCH + c_i, minlength=NBLK * NCH)
        counts[i] = cnt.reshape(NBLK, NCH)

    Mmat = np.ceil(counts.max(axis=0) / 128).astype(np.int64)  # [NBLK, NCH]
    totcol = int(Mmat.sum())

    # column-start offset of each key slot in the concatenated (key-order)
    # layout; key(b, c) = ((b//SBK)*NCH + c)*SBK + (b%SBK)
    flatM = np.zeros(NBLK * NCH, dtype=np.int64)
    for b in range(NBLK):
        for c in range(NCH):
            flatM[_key(b, c)] = Mmat[b][c]
    colstart = np.zeros(NBLK * NCH, dtype=np.int64)
    colstart[1:] = np.cumsum(flatM)[:-1]

    srcs_list, dstl_list, xsT_list = [], [], []
    for i in range(M):
        li, sl, key_s = percore[i]
        # position of each edge inside its key group
        kcnt = np.bincount(key_s, minlength=NBLK * NCH)
        kstart = np.zeros(NBLK * NCH, dtype=np.int64)
        kstart[1:] = np.cumsum(kcnt)[:-1]
        pos = np.arange(len(li)) - kstart[key_s]
        gpos = colstart[key_s] * 128 + pos  # position in padded edge stream

        v = np.zeros(totcol * 128, dtype=np.int16)   # pad: row 0 of chunk
        d = np.full(totcol * 128, -1.0, dtype=np.float16)
        v[gpos] = li
        d[gpos] = sl

        # idx stream wraps per 16 within each call; calls are whole columns
        # and 128 % 16 == 0, so the wrap is identical whether done per call
        # or over the whole stream.
        w = v.reshape(-1, 16).T.copy()               # [16, totcol*8]
        srcs_list.append(np.tile(w, (8, 1)))
        dstl_list.append(
            np.ascontiguousarray(d.reshape(totcol, 128).T)  # [128, totcol]
        )
        xs = np.zeros((P, SLOTS), dtype=np.float32)
        xs[:, pos_list[i]] = ((1.0 + epsv) * x[i * NPC:(i + 1) * NPC]).T
        xsT_list.append(xs)

    # pair table: row i holds [x[i], x[i+1]] so each 512B descriptor fetches
    # the needed row in its first half
    xpad = np.zeros((NCH * CH + 1, C), dtype=np.float16)
    xpad[:N] = x
    xt = np.concatenate([xpad[:-1], xpad[1:]], axis=1)
    return Mmat, srcs_list, dstl_list, xsT_list, xt, pos_list


def _balance(deg, nbins, cap_slots, cap_edges):
    """Best-fit-decreasing: assign dsts (rows of deg [ND, NCH]) to nbins
    blocks, <= cap_slots dsts and (soft) <= cap_edges[b, c] edges each."""
    nd = deg.shape[0]
    tot = deg.sum(axis=1)
    order = np.argsort(-tot, kind="stable")
    sums = np.zeros((nbins, deg.shape[1]), dtype=np.int64)
    load = np.zeros(nbins, dtype=np.int64)
    cnt = np.zeros(nbins, dtype=np.int64)
    blk = np.empty(nd, dtype=np.int64)
    slot = np.empty(nd, dtype=np.int64)
    big = 1 << 50
    for d in order:
        v = deg[d]
        ok = (cnt < cap_slots) & ((sums + v) <= cap_edges).all(axis=1)
        if ok.any():
            b = int(np.argmin(np.where(ok, load, big)))  # LPT: least-loaded fit
        else:
            over = np.maximum(sums + v - cap_edges, 0).sum(axis=1)
            over[cnt >= cap_slots] = big
            b = int(np.argmin(over))
        blk[d] = b
        slot[d] = cnt[b]
        cnt[b] += 1
        load[b] += tot[d]
        sums[b] += v
    return blk, slot


def make_in_maps(inputs):
    Mmat, srcs_list, dstl_list, xsT_list, xt, pos_list = prep(
        inputs["x"], inputs["edge_src"], inputs["edge_dst"], inputs["eps"]
    )
    w1 = np.ascontiguousarray(np.asarray(inputs["W1"], dtype=np.float32))
    w2 = np.ascontiguousarray(np.asarray(inputs["W2"], dtype=np.float32))
    b1c = np.asarray(inputs["b1"], dtype=np.float32).reshape(C, 1)
    b2c = np.asarray(inputs["b2"], dtype=np.float32).reshape(C, 1)
    biasc = np.asarray(inputs["bias"], dtype=np.float32).reshape(C, 1)
    iota = np.tile(np.arange(BLK, dtype=np.float16), (P, 1))
    in_maps = [
        dict(
            xt=xt, srcs=srcs_list[i], dstl=dstl_list[i], xsT=xsT_list[i],
            W1=w1, W2=w2, b1c=b1c, b2c=b2c, biasc=biasc, iota=iota,
        )
        for i in range(M)
    ]
    return Mmat, in_maps, pos_list


def get_program(Mmat):
    key = Mmat.tobytes()
    if key not in _cache:
        _cache[key] = build(Mmat)
    return _cache[key]


def assemble(results, pos_list):
    out = np.empty((N, C), dtype=np.float32)
    for i in range(M):
        out[i * NPC:(i + 1) * NPC] = results[i]["outT"].T[pos_list[i]]
    return out


def kernel(**inputs) -> np.ndarray:
    Mmat, in_maps, pos_list = make_in_maps(inputs)
    nc = get_program(Mmat)
    last_err = None
    for _ in range(3):  # rare transient NRT_EXEC_UNIT_UNRECOVERABLE flakes
        try:
            res = run_bass_kernel_spmd(nc, in_maps, list(range(M)))
            return assemble(res.results, pos_list)
        except Exception as e:  # noqa: BLE001
            last_err = e
    raise last_err
